# revision 9
# baseline (speedup 1.0000x reference)
"""Trainium2 Bass kernel for nn_CaptureCityHead (2-layer transformer + score head).

Strategy (8 NeuronCores, sequence-parallel):
  - Host gathers x = node_emb[tile_ids], precomputes RoPE cos/sin tables,
    shards rows S=2048 -> 8 x 256, ships activations TRANSPOSED [D, 256].
  - Each core computes q/k/v for its shard; k^T and v (65-col head-packed,
    with a ones column for the softmax denominator) are all-gathered across
    the 8 cores in bf16 so every core attends over the full sequence.
  - Activations stay transposed [D(part), m(free)]; weights are used
    as-stored as matmul stationary operands. Cross-partition reductions
    (LayerNorm stats, softmax denominator broadcast) run on the PE via
    ones-vector matmuls.
  - Matmul operands bf16 (fp32 PSUM accumulation); residual stream fp32.
"""
import contextlib

import numpy as np
import ml_dtypes

import concourse.bass as bass
import concourse.mybir as mybir
import concourse.tile as tile
from concourse import bacc
from concourse.bass_utils import run_bass_kernel_spmd

BF = mybir.dt.bfloat16
F32 = mybir.dt.float32
AF = mybir.ActivationFunctionType
OP = mybir.AluOpType

NC_ = 8          # cores
S, D, H, HD, L, MH = 2048, 1024, 16, 64, 2, 1024
SL = S // NC_    # 256 rows per core
PT = D // 128    # 8 partition tiles
LN_EPS = 1e-5
NPBF = ml_dtypes.bfloat16


def build(debug: bool = False):
    nc = bacc.Bacc("TRN2", target_bir_lowering=False, debug=False, num_devices=NC_)

    # ---------------- DRAM I/O ----------------
    xT_d = nc.dram_tensor("xT", [D, SL], F32, kind="ExternalInput")
    rp_d = nc.dram_tensor("ropeT", [D, SL], F32, kind="ExternalInput")
    wq_d = nc.dram_tensor("wq", [L, D, D], BF, kind="ExternalInput")
    wk_d = nc.dram_tensor("wk", [L, D, D], BF, kind="ExternalInput")
    wv_d = nc.dram_tensor("wv", [L, D, D], BF, kind="ExternalInput")
    wo_d = nc.dram_tensor("wo", [L, D, D], BF, kind="ExternalInput")
    w1_d = nc.dram_tensor("w1", [L, D, 2 * D], BF, kind="ExternalInput")
    w2_d = nc.dram_tensor("w2", [L, 2 * D, D], BF, kind="ExternalInput")
    bq_d = nc.dram_tensor("bq", [L, D], F32, kind="ExternalInput")
    bk_d = nc.dram_tensor("bk", [L, D], F32, kind="ExternalInput")
    boe_d = nc.dram_tensor("boe", [L, D], F32, kind="ExternalInput")
    b1_d = nc.dram_tensor("b1", [L, 2 * D], F32, kind="ExternalInput")
    b2_d = nc.dram_tensor("b2", [L, D], F32, kind="ExternalInput")
    g1_d = nc.dram_tensor("g1", [L, D], F32, kind="ExternalInput")
    be1_d = nc.dram_tensor("be1", [L, D], F32, kind="ExternalInput")
    g2_d = nc.dram_tensor("g2", [L, D], F32, kind="ExternalInput")
    be2_d = nc.dram_tensor("be2", [L, D], F32, kind="ExternalInput")
    ws0_d = nc.dram_tensor("ws0", [D, MH], BF, kind="ExternalInput")
    ws1_d = nc.dram_tensor("ws1", [MH, MH], BF, kind="ExternalInput")
    ws2_d = nc.dram_tensor("ws2", [MH, 1], BF, kind="ExternalInput")
    bs0_d = nc.dram_tensor("bs0", [MH], F32, kind="ExternalInput")
    bs1_d = nc.dram_tensor("bs1", [MH], F32, kind="ExternalInput")
    bs2_d = nc.dram_tensor("bs2", [1, 1], F32, kind="ExternalInput")
    out_d = nc.dram_tensor("logits", [1, SL], F32, kind="ExternalOutput")
    dbg_d = {}
    if debug:
        dbg_d["xr0"] = nc.dram_tensor("dbg_xr0", [D, SL], BF, kind="ExternalOutput")
        dbg_d["q0"] = nc.dram_tensor("dbg_q0", [D, SL], BF, kind="ExternalOutput")
        dbg_d["ctx0"] = nc.dram_tensor("dbg_ctx0", [D, SL], BF, kind="ExternalOutput")
        dbg_d["x1"] = nc.dram_tensor("dbg_x1", [D, SL], F32, kind="ExternalOutput")
        dbg_d["xl1"] = nc.dram_tensor("dbg_xl1", [D, SL], F32, kind="ExternalOutput")

    with tile.TileContext(nc) as tc:
        ctx = contextlib.ExitStack()
        with ctx:
            dramp = ctx.enter_context(tc.tile_pool(name="dramp", bufs=1, space="DRAM"))
            # collective buffers (per layer)
            kag_in, kag_out, vag_in, vag_out = [], [], [], []
            for l in range(L):
                kag_in.append(dramp.tile([D, SL], BF, tag=f"kag_in{l}",
                                         name=f"kag_in{l}"))
                kag_out.append(dramp.tile([NC_ * D, SL], BF, addr_space="Shared",
                                          tag=f"kag_out{l}", name=f"kag_out{l}"))
                vag_in.append(dramp.tile([SL, H * 65], BF, tag=f"vag_in{l}",
                                         name=f"vag_in{l}"))
                vag_out.append(dramp.tile([S, H * 65], BF, addr_space="Shared",
                                          tag=f"vag_out{l}", name=f"vag_out{l}"))
            persist = ctx.enter_context(tc.tile_pool(name="persist", bufs=1))
            sb = ctx.enter_context(tc.tile_pool(name="sb", bufs=1))
            psum = ctx.enter_context(tc.tile_pool(name="psum", bufs=2, space="PSUM"))

            def sbt(shape, dtype, tag, name, bufs):
                return sb.tile(shape, dtype, tag=tag, name=name, bufs=bufs)

            # ---------------- persistent/setup ----------------
            ones_sq = persist.tile([128, 128], F32, tag="ones_sq", name="ones_sq")
            nc.vector.memset(ones_sq[:], 1.0)
            ones_col = ones_sq[:, 0:1]       # [128, 1]
            ones_row = ones_sq[0:1, :]       # [1, 128]
            eps_t = persist.tile([1, 1], F32, tag="eps_t", name="eps_t")
            nc.vector.memset(eps_t[:], LN_EPS)

            xt = [persist.tile([128, SL], F32, tag=f"xt{i}", name=f"xt{i}")
                  for i in range(PT)]
            rp = [persist.tile([128, SL], F32, tag=f"rp{i}", name=f"rp{i}")
                  for i in range(PT)]
            for i in range(PT):
                nc.sync.dma_start(out=xt[i][:], in_=xT_d.ap()[i * 128:(i + 1) * 128, :])
                nc.sync.dma_start(out=rp[i][:], in_=rp_d.ap()[i * 128:(i + 1) * 128, :])

            # biases as per-partition columns
            def load_cols(src, shape, tag, pattern, **kw):
                t = persist.tile(shape, F32, tag=tag, name=tag)
                nc.sync.dma_start(out=t[:], in_=src.ap().rearrange(pattern, **kw))
                return t

            bq_sb = load_cols(bq_d, [128, L, PT], "bq_sb", "l (o p) -> p l o", p=128)
            bk_sb = load_cols(bk_d, [128, L, PT], "bk_sb", "l (o p) -> p l o", p=128)
            boe_sb = load_cols(boe_d, [128, L, PT], "boe_sb", "l (o p) -> p l o", p=128)
            b2_sb = load_cols(b2_d, [128, L, PT], "b2_sb", "l (o p) -> p l o", p=128)
            b1_sb = load_cols(b1_d, [128, L, 16], "b1_sb", "l (o p) -> p l o", p=128)
            g1_sb = load_cols(g1_d, [128, L, PT], "g1_sb", "l (o p) -> p l o", p=128)
            be1_sb = load_cols(be1_d, [128, L, PT], "be1_sb", "l (o p) -> p l o", p=128)
            g2_sb = load_cols(g2_d, [128, L, PT], "g2_sb", "l (o p) -> p l o", p=128)
            be2_sb = load_cols(be2_d, [128, L, PT], "be2_sb", "l (o p) -> p l o", p=128)
            bs0_sb = load_cols(bs0_d, [128, PT], "bs0_sb", "(o p) -> p o", p=128)
            bs1_sb = load_cols(bs1_d, [128, PT], "bs1_sb", "(o p) -> p o", p=128)
            bs2_sb = persist.tile([1, 1], F32, tag="bs2_sb", name="bs2_sb")
            nc.sync.dma_start(out=bs2_sb[:], in_=bs2_d.ap())
            ws2_sb = persist.tile([128, PT], BF, tag="ws2_sb", name="ws2_sb")
            nc.sync.dma_start(out=ws2_sb[:],
                              in_=ws2_d.ap().rearrange("(c p) o -> p (c o)", p=128))

            def load_w8(src_ap, nm):
                """Load a [1024, n] weight as 8 [128, n] tiles (shared 'wts' tag)."""
                tiles = []
                n = src_ap.shape[1]
                for p in range(PT):
                    t = sbt([128, n], BF, "wts", f"w_{nm}_{p}", 18)
                    nc.sync.dma_start(out=t[:], in_=src_ap[p * 128:(p + 1) * 128, :])
                    tiles.append(t)
                return tiles

            def proj_T(w_tiles, rhs_tiles, bias_col, out_tag, relu=False, bufs=9):
                """yT[o] = act(sum_p w[p][:, o*128] ^T @ rhs[p] + bias[:, o]) -> bf16."""
                outs = []
                for o in range(PT):
                    ps = psum.tile([128, SL], F32, tag="pmm", name=f"ps_{out_tag}{o}")
                    for p in range(PT):
                        nc.tensor.matmul(
                            ps[:], w_tiles[p][:, o * 128:(o + 1) * 128], rhs_tiles[p][:],
                            start=(p == 0), stop=(p == PT - 1))
                    t = sbt([128, SL], BF, out_tag, f"{out_tag}{o}", bufs)
                    if relu:
                        nc.scalar.activation(t[:], ps[:], AF.Relu, bias=bias_col(o),
                                             scale=1.0)
                    else:
                        nc.vector.tensor_scalar_add(t[:], ps[:], bias_col(o))
                    outs.append(t)
                return outs

            # ================= layers =================
            for l in range(L):
                # --- rope: xr (bf16) ---
                # XR0 = X0*R0 - X2*R2 ; XR1 = X1*R1 - X3*R3
                # XR2 = X0*R2 + X2*R0 ; XR3 = X1*R3 + X3*R1
                # XR4 = X4*R4 - X6*R6 ; XR5 = X5*R5 - X7*R7
                # XR6 = X4*R6 + X6*R4 ; XR7 = X5*R7 + X7*R5
                rope_map = [
                    (0, 0, 2, 2, -1), (1, 1, 3, 3, -1),
                    (0, 2, 2, 0, +1), (1, 3, 3, 1, +1),
                    (4, 4, 6, 6, -1), (5, 5, 7, 7, -1),
                    (4, 6, 6, 4, +1), (5, 7, 7, 5, +1),
                ]
                xr = []
                for i in range(PT):
                    xa, ra, xb_, rb, sign = rope_map[i]
                    t1 = sbt([128, SL], F32, "ropet1", f"rt1_{l}_{i}", 3)
                    t2 = sbt([128, SL], F32, "ropet2", f"rt2_{l}_{i}", 3)
                    nc.vector.tensor_mul(t1[:], xt[xa][:], rp[ra][:])
                    nc.vector.tensor_mul(t2[:], xt[xb_][:], rp[rb][:])
                    xo = sbt([128, SL], BF, "xr", f"xr{l}_{i}", 9)
                    if sign > 0:
                        nc.vector.tensor_add(xo[:], t1[:], t2[:])
                    else:
                        nc.vector.tensor_sub(xo[:], t1[:], t2[:])
                    xr.append(xo)
                if debug and l == 0:
                    for i in range(PT):
                        nc.sync.dma_start(
                            out=dbg_d["xr0"].ap()[i * 128:(i + 1) * 128, :], in_=xr[i][:])

                # --- k projection (transposed out) + AG ---
                wk_t = load_w8(wk_d.ap()[l], f"wk{l}")
                kt = proj_T(wk_t, xr, lambda o: bk_sb[:, l, o:o + 1], "kt", bufs=4)
                for o in range(PT):
                    nc.sync.dma_start(out=kag_in[l][o * 128:(o + 1) * 128, :], in_=kt[o][:])
                nc.gpsimd.collective_compute(
                    "AllGather", OP.bypass,
                    replica_groups=[list(range(NC_))],
                    ins=[kag_in[l][:]], outs=[kag_out[l][:]])

                # --- v projection (natural layout, 65-col head packing) + AG ---
                wv_t = load_w8(wv_d.ap()[l], f"wv{l}")
                xb = []
                for i in range(PT):
                    t = sbt([128, SL], BF, "xb", f"xb{l}_{i}", 9)
                    nc.vector.tensor_copy(t[:], xt[i][:])
                    xb.append(t)
                for mt in range(2):
                    va_loc = sbt([128, H * 65], BF, "va_loc", f"va_loc{l}_{mt}", 3)
                    nc.vector.memset(
                        va_loc[:].rearrange("p (h k) -> p h k", k=65)[:, :, 64:65], 1.0)
                    for nch in range(2):
                        ps = psum.tile([128, 512], F32, tag="pmm", name=f"psv{l}{mt}{nch}")
                        for p in range(PT):
                            nc.tensor.matmul(
                                ps[:],
                                xb[p][:, mt * 128:(mt + 1) * 128],
                                wv_t[p][:, nch * 512:(nch + 1) * 512],
                                start=(p == 0), stop=(p == PT - 1))
                        dst = va_loc[:].rearrange("p (h k) -> p h k", k=65)[
                            :, nch * 8:(nch + 1) * 8, 0:64]
                        src = ps[:].rearrange("p (h k) -> p h k", k=64)
                        nc.scalar.activation(dst, src, AF.Copy, scale=1.0)
                    nc.sync.dma_start(
                        out=vag_in[l][mt * 128:(mt + 1) * 128, :], in_=va_loc[:])
                nc.gpsimd.collective_compute(
                    "AllGather", OP.bypass,
                    replica_groups=[list(range(NC_))],
                    ins=[vag_in[l][:]], outs=[vag_out[l][:]])

                # --- q projection ---
                wq_t = load_w8(wq_d.ap()[l], f"wq{l}")
                qt = proj_T(wq_t, xr, lambda o: bq_sb[:, l, o:o + 1], "qt", bufs=9)
                if debug and l == 0:
                    for i in range(PT):
                        nc.sync.dma_start(
                            out=dbg_d["q0"].ap()[i * 128:(i + 1) * 128, :], in_=qt[i][:])

                # --- load V (all heads packed) ---
                va_l = []
                for j in range(16):
                    t = sbt([128, H * 65], BF, "va_l", f"va{l}_{j}", 17)
                    nc.sync.dma_start(out=t[:], in_=vag_out[l][j * 128:(j + 1) * 128, :])
                    va_l.append(t)

                # --- attention per head-pair ---
                ctxT = [sbt([128, SL], BF, "ctxT", f"ctxT{l}_{i}", 9) for i in range(PT)]
                kag_v = kag_out[l][:].rearrange("(c d) m -> c d m", c=NC_)
                for pt_i in range(PT):
                    ktp_t = []
                    for jc in range(NC_):
                        t = sbt([128, SL], BF, "ktp", f"ktp{l}_{pt_i}_{jc}", 9)
                        nc.sync.dma_start(
                            out=t[:], in_=kag_v[jc, pt_i * 128:(pt_i + 1) * 128, :])
                        ktp_t.append(t)
                    for half in range(2):
                        h = 2 * pt_i + half
                        qh = qt[pt_i][half * 64:(half + 1) * 64, :]
                        ctx_ps = psum.tile([65, SL], F32, tag="pctx", name=f"ctxps{l}_{h}")
                        for j in range(16):
                            jc, jm = j // 2, j % 2
                            sc_ps = psum.tile([128, SL], F32, tag="psc",
                                              name=f"sc{l}_{h}_{j}")
                            nc.tensor.matmul(
                                sc_ps[:],
                                ktp_t[jc][half * 64:(half + 1) * 64,
                                          jm * 128:(jm + 1) * 128],
                                qh,
                                start=True, stop=True)
                            ex = sbt([128, SL], BF, "exp", f"ex{l}_{h}_{j}", 4)
                            nc.scalar.activation(ex[:], sc_ps[:], AF.Exp, scale=0.125)
                            nc.tensor.matmul(
                                ctx_ps[:],
                                va_l[j][:, h * 65:(h + 1) * 65],
                                ex[:],
                                start=(j == 0), stop=(j == 15))
                        den = sbt([65, SL], F32, "den", f"den{l}_{h}", 3)
                        nc.vector.tensor_copy(den[64:65, :], ctx_ps[64:65, :])
                        nc.vector.reciprocal(den[64:65, :], den[64:65, :])
                        bc_ps = psum.tile([64, SL], F32, tag="pbc", name=f"bc{l}_{h}")
                        nc.tensor.matmul(bc_ps[:], ones_sq[64:65, 0:64], den[64:65, :],
                                         start=True, stop=True)
                        bc_sb = sbt([64, SL], F32, "bcs", f"bcs{l}_{h}", 3)
                        nc.scalar.mul(bc_sb[:], bc_ps[:], 1.0)
                        cn = sbt([64, SL], BF, "cn", f"cn{l}_{h}", 3)
                        nc.vector.tensor_mul(cn[:], ctx_ps[0:64, :], bc_sb[:])
                        nc.sync.dma_start(
                            out=ctxT[pt_i][half * 64:(half + 1) * 64, :], in_=cn[:])
                if debug and l == 0:
                    for i in range(PT):
                        nc.sync.dma_start(
                            out=dbg_d["ctx0"].ap()[i * 128:(i + 1) * 128, :],
                            in_=ctxT[i][:])

                # --- out projection + residual into xt ---
                wo_t = load_w8(wo_d.ap()[l], f"wo{l}")
                for o in range(PT):
                    ps = psum.tile([128, SL], F32, tag="pmm", name=f"pso{l}_{o}")
                    for p in range(PT):
                        nc.tensor.matmul(
                            ps[:], wo_t[p][:, o * 128:(o + 1) * 128], ctxT[p][:],
                            start=(p == 0), stop=(p == PT - 1))
                    tmp = sbt([128, SL], F32, "evac", f"evo{l}_{o}", 3)
                    nc.vector.tensor_scalar_add(tmp[:], ps[:], boe_sb[:, l, o:o + 1])
                    nc.vector.tensor_add(xt[o][:], tmp[:], xt[o][:])

                # --- LN (in place on xt) ---
                def layer_norm(g_col, be_col):
                    mean_ps = psum.tile([1, SL], F32, tag="pbc", name="mean_ps")
                    for o in range(PT):
                        nc.tensor.matmul(mean_ps[:], ones_col[:], xt[o][:],
                                         start=(o == 0), stop=(o == PT - 1))
                    sq_ps = psum.tile([1, SL], F32, tag="pbc", name="sq_ps")
                    for o in range(PT):
                        sq = sbt([128, SL], F32, "sq", f"sq{o}", 3)
                        nc.vector.tensor_mul(sq[:], xt[o][:], xt[o][:])
                        nc.tensor.matmul(sq_ps[:], ones_col[:], sq[:],
                                         start=(o == 0), stop=(o == PT - 1))
                    mean = sbt([1, SL], F32, "lnm", "mean", 2)
                    nc.scalar.mul(mean[:], mean_ps[:], 1.0 / D)
                    m2 = sbt([1, SL], F32, "lnm2", "m2", 2)
                    nc.vector.tensor_mul(m2[:], mean[:], mean[:])
                    var = sbt([1, SL], F32, "lnv", "var", 2)
                    nc.scalar.mul(var[:], sq_ps[:], 1.0 / D)
                    var2 = sbt([1, SL], F32, "lnv2", "var2", 2)
                    nc.vector.tensor_sub(var2[:], var[:], m2[:])
                    std = sbt([1, SL], F32, "lnstd", "std", 2)
                    nc.scalar.activation(std[:], var2[:], AF.Sqrt, bias=eps_t[0:1, 0:1],
                                         scale=1.0)
                    rstd = sbt([1, SL], F32, "lnr", "rstd", 2)
                    nc.vector.reciprocal(rstd[:], std[:])
                    prem = sbt([1, SL], F32, "lnp", "prem", 2)
                    nc.vector.tensor_mul(prem[:], mean[:], rstd[:])
                    rstd_b = psum.tile([128, SL], F32, tag="pbc", name="rstd_b")
                    nc.tensor.matmul(rstd_b[:], ones_row[:], rstd[:], start=True, stop=True)
                    prem_b = psum.tile([128, SL], F32, tag="pbc", name="prem_b")
                    nc.tensor.matmul(prem_b[:], ones_row[:], prem[:], start=True, stop=True)
                    for o in range(PT):
                        t1 = sbt([128, SL], F32, "lnt1", f"lnt1_{o}", 3)
                        nc.vector.tensor_mul(t1[:], xt[o][:], rstd_b[:])
                        t2 = sbt([128, SL], F32, "lnt2", f"lnt2_{o}", 3)
                        nc.vector.tensor_sub(t2[:], t1[:], prem_b[:])
                        nc.vector.tensor_scalar(
                            out=xt[o][:], in0=t2[:], scalar1=g_col(o), scalar2=be_col(o),
                            op0=OP.mult, op1=OP.add)

                layer_norm(lambda o: g1_sb[:, l, o:o + 1], lambda o: be1_sb[:, l, o:o + 1])
                x1b = []
                for o in range(PT):
                    t = sbt([128, SL], BF, "x1b", f"x1b{l}_{o}", 9)
                    nc.vector.tensor_copy(t[:], xt[o][:])
                    x1b.append(t)
                if debug and l == 0:
                    for i in range(PT):
                        nc.sync.dma_start(
                            out=dbg_d["x1"].ap()[i * 128:(i + 1) * 128, :], in_=xt[i][:])

                # --- FFN ---
                w1a_t = load_w8(w1_d.ap()[l][:, 0:D], f"w1a{l}")
                w1b_t = load_w8(w1_d.ap()[l][:, D:2 * D], f"w1b{l}")
                h1b = []
                for ho in range(16):
                    wt_t = w1a_t if ho < 8 else w1b_t
                    oo = ho % 8
                    ps = psum.tile([128, SL], F32, tag="pmm", name=f"psf{l}_{ho}")
                    for p in range(PT):
                        nc.tensor.matmul(
                            ps[:], wt_t[p][:, oo * 128:(oo + 1) * 128], x1b[p][:],
                            start=(p == 0), stop=(p == PT - 1))
                    hb = sbt([128, SL], BF, "h1b", f"h1b{l}_{ho}", 18)
                    nc.scalar.activation(hb[:], ps[:], AF.Relu,
                                         bias=b1_sb[:, l, ho:ho + 1], scale=1.0)
                    h1b.append(hb)
                w2_t = []
                for hc in range(16):
                    t = sbt([128, D], BF, "wts", f"w2_{l}_{hc}", 18)
                    nc.sync.dma_start(out=t[:],
                                      in_=w2_d.ap()[l][hc * 128:(hc + 1) * 128, :])
                    w2_t.append(t)
                for o in range(PT):
                    ps = psum.tile([128, SL], F32, tag="pmm", name=f"psf2{l}_{o}")
                    for hc in range(16):
                        nc.tensor.matmul(
                            ps[:], w2_t[hc][:, o * 128:(o + 1) * 128], h1b[hc][:],
                            start=(hc == 0), stop=(hc == 15))
                    tmp = sbt([128, SL], F32, "evac", f"evf{l}_{o}", 3)
                    nc.vector.tensor_scalar_add(tmp[:], ps[:], b2_sb[:, l, o:o + 1])
                    nc.vector.tensor_add(xt[o][:], tmp[:], xt[o][:])

                layer_norm(lambda o: g2_sb[:, l, o:o + 1], lambda o: be2_sb[:, l, o:o + 1])
                if debug and l == 0:
                    for i in range(PT):
                        nc.sync.dma_start(
                            out=dbg_d["xl1"].ap()[i * 128:(i + 1) * 128, :], in_=xt[i][:])

            # ================= head MLP =================
            xb2 = []
            for i in range(PT):
                t = sbt([128, SL], BF, "xb2", f"xb2_{i}", 9)
                nc.vector.tensor_copy(t[:], xt[i][:])
                xb2.append(t)
            ws0_t = load_w8(ws0_d.ap(), "ws0")
            h0 = proj_T(ws0_t, xb2, lambda o: bs0_sb[:, o:o + 1], "h0", relu=True)
            ws1_t = load_w8(ws1_d.ap(), "ws1")
            h1 = proj_T(ws1_t, h0, lambda o: bs1_sb[:, o:o + 1], "h1", relu=True)
            lg_ps = psum.tile([1, SL], F32, tag="pbc", name="lg_ps")
            for p in range(PT):
                nc.tensor.matmul(lg_ps[:], ws2_sb[:, p:p + 1], h1[p][:],
                                 start=(p == 0), stop=(p == PT - 1))
            lg = sbt([1, SL], F32, "lg", "lg", 2)
            nc.vector.tensor_scalar_add(lg[:], lg_ps[:], bs2_sb[0:1, 0:1])
            nc.sync.dma_start(out=out_d.ap(), in_=lg[:])

    nc.compile()
    return nc


# ---------------- host side ----------------
_BUILT = {}


def _get_built(debug=False):
    key = bool(debug)
    if key not in _BUILT:
        _BUILT[key] = build(debug=debug)
    return _BUILT[key]


def _host_prep(inputs):
    inp = {k: (np.asarray(v) if not np.isscalar(v) else v) for k, v in inputs.items()}
    tile_ids = np.asarray(inp["tile_ids"]).astype(np.int64)
    Ny = int(np.asarray(inp["Ny"]))
    node_emb = np.asarray(inp["node_emb"], dtype=np.float32)
    x0 = node_emb[tile_ids]                       # [S, D]

    hh = 256
    theta = (1.0 / (10000.0 ** (np.arange(hh, dtype=np.float32) / hh))).astype(np.float32)
    rows = (tile_ids // Ny).astype(np.float32)
    cols = (tile_ids % Ny).astype(np.float32)
    ang_r = rows[:, None] * theta[None, :]
    ang_c = cols[:, None] * theta[None, :]
    cr, sr = np.cos(ang_r), np.sin(ang_r)
    cc, sc = np.cos(ang_c), np.sin(ang_c)

    def bf(x):
        return np.ascontiguousarray(np.asarray(x, dtype=np.float32)).astype(NPBF)

    def f32(x):
        return np.ascontiguousarray(np.asarray(x, dtype=np.float32))

    shared = {
        "wq": bf(inp["wq"]), "wk": bf(inp["wk"]), "wv": bf(inp["wv"]),
        "wo": bf(inp["wo"]), "w1": bf(inp["w1"]), "w2": bf(inp["w2"]),
        "bq": f32(inp["bq"]), "bk": f32(inp["bk"]),
        "b1": f32(inp["b1"]), "b2": f32(inp["b2"]),
        "g1": f32(inp["g1"]), "be1": f32(inp["be1"]),
        "g2": f32(inp["g2"]), "be2": f32(inp["be2"]),
        "ws0": bf(inp["ws0"]), "ws1": bf(inp["ws1"]), "ws2": bf(inp["ws2"]),
        "bs0": f32(inp["bs0"]), "bs1": f32(inp["bs1"]),
        "bs2": f32(inp["bs2"]).reshape(1, 1),
    }
    bv = f32(inp["bv"])
    wo = f32(inp["wo"])
    bo = f32(inp["bo"])
    shared["boe"] = np.stack([bv[l] @ wo[l] + bo[l] for l in range(L)]).astype(np.float32)

    in_maps = []
    for c in range(NC_):
        sl = slice(c * SL, (c + 1) * SL)
        m = dict(shared)
        m["xT"] = np.ascontiguousarray(x0[sl].T).astype(np.float32)
        m["ropeT"] = np.ascontiguousarray(
            np.concatenate([cr[sl].T, sr[sl].T, cc[sl].T, sc[sl].T], axis=0)
        ).astype(np.float32)
        in_maps.append(m)
    return in_maps


def kernel(**inputs):
    nc = _get_built(debug=False)
    in_maps = _host_prep(inputs)
    res = run_bass_kernel_spmd(nc, in_maps, core_ids=list(range(NC_)))
    logits = np.concatenate(
        [np.asarray(res.results[c]["logits"]).reshape(SL) for c in range(NC_)])
    return logits.astype(np.float32)


if __name__ == "__main__":
    data = np.load("/root/problem/ref_data.npz")
    expected = data["__expected"]
    inputs = {k: data[k] for k in data.files if k != "__expected"}
    got = kernel(**inputs)
    err = np.abs(got - expected)
    rel = np.linalg.norm(got - expected) / np.linalg.norm(expected)
    print("max abs err:", err.max(), "rel l2:", rel)


# revision 12
# speedup vs baseline: 1.1016x; 1.1016x over previous
"""Trainium2 Bass kernel for nn_CaptureCityHead (2-layer transformer + score head).

Strategy (8 NeuronCores, sequence-parallel):
  - Host gathers x = node_emb[tile_ids], precomputes RoPE cos/sin tables,
    shards rows S=2048 -> 8 x 256, ships activations TRANSPOSED [D, 256].
  - Each core computes q/k/v for its shard; k^T and v (65-col head-packed,
    with a ones column for the softmax denominator) are all-gathered across
    the 8 cores in bf16. AGs are split into head-halves (A: heads 0-7,
    B: heads 8-15) so attention on A overlaps the B gathers; a dummy
    all-gather at kernel start absorbs the one-time collective barrier.
  - Activations stay transposed [D(part), m(free)]; weights are used
    as-stored as matmul stationary operands. Cross-partition reductions
    (LayerNorm stats, softmax denominator broadcast) run on the PE via
    ones-vector matmuls.
  - Matmul operands bf16 (fp32 PSUM accumulation); residual stream fp32.
"""
import contextlib

import numpy as np
import ml_dtypes

import concourse.bass as bass
import concourse.mybir as mybir
import concourse.tile as tile
from concourse import bacc
from concourse.bass_utils import run_bass_kernel_spmd

BF = mybir.dt.bfloat16
F32 = mybir.dt.float32
AF = mybir.ActivationFunctionType
OP = mybir.AluOpType

NC_ = 8          # cores
S, D, H, HD, L, MH = 2048, 1024, 16, 64, 2, 1024
SL = S // NC_    # 256 rows per core
PT = D // 128    # 8 partition tiles
LN_EPS = 1e-5
NPBF = ml_dtypes.bfloat16


def build():
    nc = bacc.Bacc("TRN2", target_bir_lowering=False, debug=False, num_devices=NC_)

    # ---------------- DRAM I/O ----------------
    xT_d = nc.dram_tensor("xT", [D, SL], F32, kind="ExternalInput")
    rp_d = nc.dram_tensor("ropeT", [D, SL], F32, kind="ExternalInput")
    bp_d = nc.dram_tensor("biasp", [128, 177], F32, kind="ExternalInput")
    wq_d = nc.dram_tensor("wq", [L, D, D], BF, kind="ExternalInput")
    wk_d = nc.dram_tensor("wk", [L, D, D], BF, kind="ExternalInput")
    wv_d = nc.dram_tensor("wv", [L, D, D], BF, kind="ExternalInput")
    wo_d = nc.dram_tensor("wo", [L, D, D], BF, kind="ExternalInput")
    w1_d = nc.dram_tensor("w1", [L, D, 2 * D], BF, kind="ExternalInput")
    w2_d = nc.dram_tensor("w2", [L, 2 * D, D], BF, kind="ExternalInput")
    ws0_d = nc.dram_tensor("ws0", [D, MH], BF, kind="ExternalInput")
    ws1_d = nc.dram_tensor("ws1", [MH, MH], BF, kind="ExternalInput")
    ws2_d = nc.dram_tensor("ws2", [MH, 1], BF, kind="ExternalInput")
    out_d = nc.dram_tensor("logits", [1, SL], F32, kind="ExternalOutput")

    with tile.TileContext(nc) as tc:
        ctx = contextlib.ExitStack()
        with ctx:
            dramp = ctx.enter_context(tc.tile_pool(name="dramp", bufs=1, space="DRAM"))
            rg = [list(range(NC_))]

            def dtile(shape, tag, shared=False):
                return dramp.tile(shape, BF, tag=tag, name=tag,
                                  addr_space="Shared" if shared else "Local")

            dum_in = dtile([1, 16], "dum_in")
            dum_out = dtile([NC_, 16], "dum_out", shared=True)
            # per layer, per half (A=heads 0-7, B=8-15)
            kag_in = [[dtile([D // 2, SL], f"kag_in{l}{g}") for g in range(2)]
                      for l in range(L)]
            kag_out = [[dtile([NC_ * D // 2, SL], f"kag_out{l}{g}", shared=True)
                        for g in range(2)] for l in range(L)]
            vag_in = [[dtile([SL, 8 * 65], f"vag_in{l}{g}") for g in range(2)]
                      for l in range(L)]
            vag_out = [[dtile([S, 8 * 65], f"vag_out{l}{g}", shared=True)
                        for g in range(2)] for l in range(L)]

            persist = ctx.enter_context(tc.tile_pool(name="persist", bufs=1))
            sb = ctx.enter_context(tc.tile_pool(name="sb", bufs=1))
            psum = ctx.enter_context(tc.tile_pool(name="psum", bufs=2, space="PSUM"))

            def sbt(shape, dtype, tag, name, bufs):
                return sb.tile(shape, dtype, tag=tag, name=name, bufs=bufs)

            # ---- dummy collective first: absorbs the one-time barrier ----
            dmy = persist.tile([1, 16], BF, tag="dmy", name="dmy")
            nc.vector.memset(dmy[:], 0.0)
            nc.sync.dma_start(out=dum_in[:], in_=dmy[:])
            nc.gpsimd.collective_compute(
                "AllGather", OP.bypass, replica_groups=rg,
                ins=[dum_in[:]], outs=[dum_out[:]])

            # ---------------- persistent/setup ----------------
            ones_sq = persist.tile([128, 128], F32, tag="ones_sq", name="ones_sq")
            nc.vector.memset(ones_sq[:], 1.0)
            ones_col = ones_sq[:, 0:1]
            ones_row = ones_sq[0:1, :]
            eps_t = persist.tile([1, 1], F32, tag="eps_t", name="eps_t")
            nc.vector.memset(eps_t[:], LN_EPS)

            xt_all = persist.tile([128, PT, SL], F32, tag="xt_all", name="xt_all")
            nc.sync.dma_start(out=xt_all[:],
                              in_=xT_d.ap().rearrange("(t p) m -> p t m", p=128))
            rp_all = persist.tile([128, PT, SL], F32, tag="rp_all", name="rp_all")
            nc.sync.dma_start(out=rp_all[:],
                              in_=rp_d.ap().rearrange("(t p) m -> p t m", p=128))

            def xt(i):
                return xt_all[:, i, :]

            def rp(i):
                return rp_all[:, i, :]

            biasp = persist.tile([128, 177], F32, tag="biasp", name="biasp")
            nc.sync.dma_start(out=biasp[:], in_=bp_d.ap())
            # col layout: bq 0-15 (l*8+o), bk 16, boe 32, b2 48, g1 64, be1 80,
            # g2 96, be2 112, b1 128-159 (l*16+ho), bs0 160, bs1 168, bs2 @ [0,176]
            bcol = {
                "bq": 0, "bk": 16, "boe": 32, "b2": 48,
                "g1": 64, "be1": 80, "g2": 96, "be2": 112,
            }

            def bc(nm, l, o):
                c = bcol[nm] + l * 8 + o
                return biasp[:, c:c + 1]

            def b1c(l, ho):
                c = 128 + l * 16 + ho
                return biasp[:, c:c + 1]

            ws2_sb = persist.tile([128, PT], BF, tag="ws2_sb", name="ws2_sb")
            nc.sync.dma_start(out=ws2_sb[:],
                              in_=ws2_d.ap().rearrange("(c p) o -> p (c o)", p=128))

            def load_wbig(src_ap, nm):
                """One [1024, 1024] weight block -> single [128, 8, 1024] tile."""
                t = sbt([128, PT, 1024], BF, "wbig", f"w_{nm}", 2)
                nc.sync.dma_start(out=t[:],
                                  in_=src_ap.rearrange("(t p) n -> p t n", p=128))
                return t

            def proj_T(w_all, rhs_fn, bias_fn, out_tag, relu=False, bufs=9,
                       o_range=range(PT)):
                """yT[o] = act(sum_p w[:, p, o*128]^T @ rhs(p) + bias(o)) -> bf16."""
                outs = []
                for o in o_range:
                    ps = psum.tile([128, SL], F32, tag="pmm", name=f"ps_{out_tag}{o}")
                    for p in range(PT):
                        nc.tensor.matmul(
                            ps[:], w_all[:, p, o * 128:(o + 1) * 128], rhs_fn(p),
                            start=(p == 0), stop=(p == PT - 1))
                    t = sbt([128, SL], BF, out_tag, f"{out_tag}{o}", bufs)
                    if relu:
                        nc.scalar.activation(t[:], ps[:], AF.Relu, bias=bias_fn(o),
                                             scale=1.0)
                    else:
                        nc.vector.tensor_scalar_add(t[:], ps[:], bias_fn(o))
                    outs.append(t)
                return outs

            # ================= layers =================
            for l in range(L):
                # --- rope: xr (bf16) ---
                rope_map = [
                    (0, 0, 2, 2, -1), (1, 1, 3, 3, -1),
                    (0, 2, 2, 0, +1), (1, 3, 3, 1, +1),
                    (4, 4, 6, 6, -1), (5, 5, 7, 7, -1),
                    (4, 6, 6, 4, +1), (5, 7, 7, 5, +1),
                ]
                xr = []
                for i in range(PT):
                    xa, ra, xb_, rb, sign = rope_map[i]
                    t1 = sbt([128, SL], F32, "ropet1", f"rt1_{l}_{i}", 3)
                    t2 = sbt([128, SL], F32, "ropet2", f"rt2_{l}_{i}", 3)
                    nc.vector.tensor_mul(t1[:], xt(xa), rp(ra))
                    nc.vector.tensor_mul(t2[:], xt(xb_), rp(rb))
                    xo = sbt([128, SL], BF, "xr", f"xr{l}_{i}", 9)
                    if sign > 0:
                        nc.vector.tensor_add(xo[:], t1[:], t2[:])
                    else:
                        nc.vector.tensor_sub(xo[:], t1[:], t2[:])
                    xr.append(xo)

                # --- k projection halves + AGs ---
                wk_t = load_wbig(wk_d.ap()[l], f"wk{l}")
                kt_a = proj_T(wk_t, lambda p: xr[p][:], lambda o: bc("bk", l, o),
                              "kt", bufs=4, o_range=range(0, 4))
                for o in range(4):
                    nc.sync.dma_start(out=kag_in[l][0][o * 128:(o + 1) * 128, :],
                                      in_=kt_a[o][:])
                nc.gpsimd.collective_compute(
                    "AllGather", OP.bypass, replica_groups=rg,
                    ins=[kag_in[l][0][:]], outs=[kag_out[l][0][:]])

                # --- v projection (natural layout, 65-col head packing) ---
                wv_t = load_wbig(wv_d.ap()[l], f"wv{l}")
                xb = []
                for i in range(PT):
                    t = sbt([128, SL], BF, "xb", f"xb{l}_{i}", 9)
                    nc.vector.tensor_copy(t[:], xt(i))
                    xb.append(t)
                va_loc = {}
                for nch in range(2):          # head group (A/B)
                    for mt in range(2):       # row half
                        vt = sbt([128, 8 * 65], BF, "va_loc", f"va_loc{l}_{nch}{mt}", 4)
                        nc.vector.memset(
                            vt[:].rearrange("p (h k) -> p h k", k=65)[:, :, 64:65], 1.0)
                        va_loc[(nch, mt)] = vt
                for mt in range(2):
                    for nch in range(2):
                        ps = psum.tile([128, 512], F32, tag="pmm", name=f"psv{l}{mt}{nch}")
                        for p in range(PT):
                            nc.tensor.matmul(
                                ps[:],
                                xb[p][:, mt * 128:(mt + 1) * 128],
                                wv_t[:, p, nch * 512:(nch + 1) * 512],
                                start=(p == 0), stop=(p == PT - 1))
                        vt = va_loc[(nch, mt)]
                        dst = vt[:].rearrange("p (h k) -> p h k", k=65)[:, :, 0:64]
                        src = ps[:].rearrange("p (h k) -> p h k", k=64)
                        nc.scalar.activation(dst, src, AF.Copy, scale=1.0)
                for nch in range(2):
                    for mt in range(2):
                        nc.sync.dma_start(
                            out=vag_in[l][nch][mt * 128:(mt + 1) * 128, :],
                            in_=va_loc[(nch, mt)][:])
                nc.gpsimd.collective_compute(
                    "AllGather", OP.bypass, replica_groups=rg,
                    ins=[vag_in[l][0][:]], outs=[vag_out[l][0][:]])

                # --- k second half + AG-B pair ---
                kt_b = proj_T(wk_t, lambda p: xr[p][:], lambda o: bc("bk", l, o),
                              "kt", bufs=4, o_range=range(4, 8))
                for o in range(4, 8):
                    nc.sync.dma_start(out=kag_in[l][1][(o - 4) * 128:(o - 3) * 128, :],
                                      in_=kt_b[o - 4][:])
                nc.gpsimd.collective_compute(
                    "AllGather", OP.bypass, replica_groups=rg,
                    ins=[kag_in[l][1][:]], outs=[kag_out[l][1][:]])
                nc.gpsimd.collective_compute(
                    "AllGather", OP.bypass, replica_groups=rg,
                    ins=[vag_in[l][1][:]], outs=[vag_out[l][1][:]])

                # --- q projection ---
                wq_t = load_wbig(wq_d.ap()[l], f"wq{l}")
                qt = proj_T(wq_t, lambda p: xr[p][:], lambda o: bc("bq", l, o),
                            "qt", bufs=9)

                # --- attention, group A then B ---
                ctxT = [sbt([128, SL], BF, "ctxT", f"ctxT{l}_{i}", 9)
                        for i in range(PT)]
                for grp in range(2):
                    # V tiles for this head group: [128, 8, 520] x2 (j 0-7, 8-15)
                    va_t = []
                    vv = vag_out[l][grp][:].rearrange("(j p) n -> p j n", p=128)
                    for jg in range(2):
                        t = sbt([128, 8, 8 * 65], BF, "va", f"va{l}_{grp}{jg}", 4)
                        nc.sync.dma_start(out=t[:], in_=vv[:, jg * 8:(jg + 1) * 8, :])
                        va_t.append(t)
                    kk = kag_out[l][grp][:].rearrange(
                        "(c t p) m -> p t c m", c=NC_, t=4, p=128)
                    for pt_i in range(4 * grp, 4 * grp + 4):
                        ktp_t = sbt([128, NC_, SL], BF, "ktp", f"ktp{l}_{pt_i}", 3)
                        nc.sync.dma_start(out=ktp_t[:], in_=kk[:, pt_i - 4 * grp, :, :])
                        for half in range(2):
                            h = 2 * pt_i + half
                            hh = h - 8 * grp
                            qh = qt[pt_i][half * 64:(half + 1) * 64, :]
                            ctx_ps = psum.tile([65, SL], F32, tag="pctx",
                                               name=f"ctxps{l}_{h}")
                            for jj in range(8):
                                sc2 = psum.tile([128, 2 * SL], F32, tag="psc",
                                                name=f"sc{l}_{h}_{jj}")
                                nc.tensor.matmul(
                                    sc2[:, 0:SL],
                                    ktp_t[half * 64:(half + 1) * 64, jj, 0:128],
                                    qh, start=True, stop=True)
                                nc.tensor.matmul(
                                    sc2[:, SL:2 * SL],
                                    ktp_t[half * 64:(half + 1) * 64, jj, 128:256],
                                    qh, start=True, stop=True)
                                ex2 = sbt([128, 2 * SL], BF, "exp", f"ex{l}_{h}_{jj}", 3)
                                nc.scalar.activation(ex2[:], sc2[:], AF.Exp, scale=0.125)
                                j0, j1 = 2 * jj, 2 * jj + 1
                                nc.tensor.matmul(
                                    ctx_ps[:],
                                    va_t[j0 // 8][:, j0 % 8, hh * 65:(hh + 1) * 65],
                                    ex2[:, 0:SL],
                                    start=(jj == 0), stop=False)
                                nc.tensor.matmul(
                                    ctx_ps[:],
                                    va_t[j1 // 8][:, j1 % 8, hh * 65:(hh + 1) * 65],
                                    ex2[:, SL:2 * SL],
                                    start=False, stop=(jj == 7))
                            den = sbt([65, SL], F32, "den", f"den{l}_{h}", 3)
                            nc.vector.tensor_copy(den[64:65, :], ctx_ps[64:65, :])
                            nc.vector.reciprocal(den[64:65, :], den[64:65, :])
                            bc_ps = psum.tile([64, SL], F32, tag="pbc", name=f"bc{l}_{h}")
                            nc.tensor.matmul(bc_ps[:], ones_sq[64:65, 0:64],
                                             den[64:65, :], start=True, stop=True)
                            bc_sb = sbt([64, SL], F32, "bcs", f"bcs{l}_{h}", 3)
                            nc.scalar.mul(bc_sb[:], bc_ps[:], 1.0)
                            cn = sbt([64, SL], BF, "cn", f"cn{l}_{h}", 3)
                            nc.vector.tensor_mul(cn[:], ctx_ps[0:64, :], bc_sb[:])
                            nc.sync.dma_start(
                                out=ctxT[pt_i][half * 64:(half + 1) * 64, :], in_=cn[:])

                # --- out projection + residual into xt ---
                wo_t = load_wbig(wo_d.ap()[l], f"wo{l}")
                for o in range(PT):
                    ps = psum.tile([128, SL], F32, tag="pmm", name=f"pso{l}_{o}")
                    for p in range(PT):
                        nc.tensor.matmul(
                            ps[:], wo_t[:, p, o * 128:(o + 1) * 128], ctxT[p][:],
                            start=(p == 0), stop=(p == PT - 1))
                    tmp = sbt([128, SL], F32, "evac", f"evo{l}_{o}", 3)
                    nc.vector.tensor_scalar_add(tmp[:], ps[:], bc("boe", l, o))
                    nc.vector.tensor_add(xt(o), tmp[:], xt(o))

                # --- LN (in place on xt) ---
                def layer_norm(g_nm, be_nm):
                    mean_ps = psum.tile([1, SL], F32, tag="pbc", name="mean_ps")
                    for o in range(PT):
                        nc.tensor.matmul(mean_ps[:], ones_col, xt(o),
                                         start=(o == 0), stop=(o == PT - 1))
                    sq_ps = psum.tile([1, SL], F32, tag="pbc", name="sq_ps")
                    for o in range(PT):
                        sq = sbt([128, SL], F32, "sq", f"sq{o}", 2)
                        nc.vector.tensor_mul(sq[:], xt(o), xt(o))
                        nc.tensor.matmul(sq_ps[:], ones_col, sq[:],
                                         start=(o == 0), stop=(o == PT - 1))
                    mean = sbt([1, SL], F32, "lnm", "mean", 2)
                    nc.scalar.mul(mean[:], mean_ps[:], 1.0 / D)
                    m2 = sbt([1, SL], F32, "lnm2", "m2", 2)
                    nc.vector.tensor_mul(m2[:], mean[:], mean[:])
                    var = sbt([1, SL], F32, "lnv", "var", 2)
                    nc.scalar.mul(var[:], sq_ps[:], 1.0 / D)
                    var2 = sbt([1, SL], F32, "lnv2", "var2", 2)
                    nc.vector.tensor_sub(var2[:], var[:], m2[:])
                    # std = sqrt(var + eps); rstd = 1/std
                    std = sbt([1, SL], F32, "lnstd", "std", 2)
                    nc.scalar.activation(std[:], var2[:], AF.Sqrt, bias=eps_t[0:1, 0:1],
                                         scale=1.0)
                    rstd = sbt([1, SL], F32, "lnr", "rstd", 2)
                    nc.vector.reciprocal(rstd[:], std[:])
                    prem = sbt([1, SL], F32, "lnp", "prem", 2)
                    nc.vector.tensor_mul(prem[:], mean[:], rstd[:])
                    rstd_b = psum.tile([128, SL], F32, tag="pbc", name="rstd_b")
                    nc.tensor.matmul(rstd_b[:], ones_row, rstd[:], start=True, stop=True)
                    prem_b = psum.tile([128, SL], F32, tag="pbc", name="prem_b")
                    nc.tensor.matmul(prem_b[:], ones_row, prem[:], start=True, stop=True)
                    for o in range(PT):
                        t1 = sbt([128, SL], F32, "lnt1", f"lnt1_{o}", 3)
                        nc.vector.tensor_mul(t1[:], xt(o), rstd_b[:])
                        t2 = sbt([128, SL], F32, "lnt2", f"lnt2_{o}", 3)
                        nc.vector.tensor_sub(t2[:], t1[:], prem_b[:])
                        nc.vector.tensor_scalar(
                            out=xt(o), in0=t2[:], scalar1=bc(g_nm, l, o),
                            scalar2=bc(be_nm, l, o), op0=OP.mult, op1=OP.add)

                layer_norm("g1", "be1")
                x1b = []
                for o in range(PT):
                    t = sbt([128, SL], BF, "x1b", f"x1b{l}_{o}", 9)
                    nc.vector.tensor_copy(t[:], xt(o))
                    x1b.append(t)

                # --- FFN ---
                w1a_t = load_wbig(w1_d.ap()[l][:, 0:D], f"w1a{l}")
                w1b_t = load_wbig(w1_d.ap()[l][:, D:2 * D], f"w1b{l}")
                h1b = []
                for ho in range(16):
                    wt_t = w1a_t if ho < 8 else w1b_t
                    oo = ho % 8
                    ps = psum.tile([128, SL], F32, tag="pmm", name=f"psf{l}_{ho}")
                    for p in range(PT):
                        nc.tensor.matmul(
                            ps[:], wt_t[:, p, oo * 128:(oo + 1) * 128], x1b[p][:],
                            start=(p == 0), stop=(p == PT - 1))
                    hb = sbt([128, SL], BF, "h1b", f"h1b{l}_{ho}", 17)
                    # relu(ps + b1) on DVE to keep ACT free for attention exp
                    nc.vector.tensor_scalar(
                        out=hb[:], in0=ps[:], scalar1=b1c(l, ho), scalar2=0.0,
                        op0=OP.add, op1=OP.max)
                    h1b.append(hb)
                w2a_t = load_wbig(w2_d.ap()[l][0:D, :], f"w2a{l}")
                w2b_t = load_wbig(w2_d.ap()[l][D:2 * D, :], f"w2b{l}")
                for o in range(PT):
                    ps = psum.tile([128, SL], F32, tag="pmm", name=f"psf2{l}_{o}")
                    for hc in range(16):
                        w_all = w2a_t if hc < 8 else w2b_t
                        nc.tensor.matmul(
                            ps[:], w_all[:, hc % 8, o * 128:(o + 1) * 128], h1b[hc][:],
                            start=(hc == 0), stop=(hc == 15))
                    tmp = sbt([128, SL], F32, "evac", f"evf{l}_{o}", 3)
                    nc.vector.tensor_scalar_add(tmp[:], ps[:], bc("b2", l, o))
                    nc.vector.tensor_add(xt(o), tmp[:], xt(o))

                layer_norm("g2", "be2")

            # ================= head MLP =================
            xb2 = []
            for i in range(PT):
                t = sbt([128, SL], BF, "xb2", f"xb2_{i}", 9)
                nc.vector.tensor_copy(t[:], xt(i))
                xb2.append(t)
            ws0_t = load_wbig(ws0_d.ap(), "ws0")
            h0 = proj_T(ws0_t, lambda p: xb2[p][:],
                        lambda o: biasp[:, 160 + o:160 + o + 1], "h0", relu=True)
            ws1_t = load_wbig(ws1_d.ap(), "ws1")
            h1 = proj_T(ws1_t, lambda p: h0[p][:],
                        lambda o: biasp[:, 168 + o:168 + o + 1], "h1", relu=True)
            lg_ps = psum.tile([1, SL], F32, tag="pbc", name="lg_ps")
            for p in range(PT):
                nc.tensor.matmul(lg_ps[:], ws2_sb[:, p:p + 1], h1[p][:],
                                 start=(p == 0), stop=(p == PT - 1))
            lg = sbt([1, SL], F32, "lg", "lg", 2)
            nc.vector.tensor_scalar_add(lg[:], lg_ps[:], biasp[0:1, 176:177])
            nc.sync.dma_start(out=out_d.ap(), in_=lg[:])

    nc.compile()
    return nc


# ---------------- host side ----------------
_BUILT = {}


def _get_built():
    if "nc" not in _BUILT:
        _BUILT["nc"] = build()
    return _BUILT["nc"]


def _host_prep(inputs):
    inp = {k: (np.asarray(v) if not np.isscalar(v) else v) for k, v in inputs.items()}
    tile_ids = np.asarray(inp["tile_ids"]).astype(np.int64)
    Ny = int(np.asarray(inp["Ny"]))
    node_emb = np.asarray(inp["node_emb"], dtype=np.float32)
    x0 = node_emb[tile_ids]                       # [S, D]

    hh = 256
    theta = (1.0 / (10000.0 ** (np.arange(hh, dtype=np.float32) / hh))).astype(np.float32)
    rows = (tile_ids // Ny).astype(np.float32)
    cols = (tile_ids % Ny).astype(np.float32)
    cr, sr = np.cos(rows[:, None] * theta[None, :]), np.sin(rows[:, None] * theta[None, :])
    cc, sc = np.cos(cols[:, None] * theta[None, :]), np.sin(cols[:, None] * theta[None, :])

    def bf(x):
        return np.ascontiguousarray(np.asarray(x, dtype=np.float32)).astype(NPBF)

    def f32(x):
        return np.ascontiguousarray(np.asarray(x, dtype=np.float32))

    # bias pack [128, 177]
    bp = np.zeros((128, 177), np.float32)

    def pack2(dst_col, arr, n):   # arr [L, n*128] -> cols dst_col + l*8(or16)+o
        a = f32(arr).reshape(L, n, 128).transpose(2, 0, 1).reshape(128, L * n)
        bp[:, dst_col:dst_col + L * n] = a

    bv = f32(inp["bv"])
    wo = f32(inp["wo"])
    bo = f32(inp["bo"])
    boe = np.stack([bv[l] @ wo[l] + bo[l] for l in range(L)]).astype(np.float32)
    pack2(0, inp["bq"], 8)
    pack2(16, inp["bk"], 8)
    pack2(32, boe, 8)
    pack2(48, inp["b2"], 8)
    pack2(64, inp["g1"], 8)
    pack2(80, inp["be1"], 8)
    pack2(96, inp["g2"], 8)
    pack2(112, inp["be2"], 8)
    pack2(128, inp["b1"], 16)
    bp[:, 160:168] = f32(inp["bs0"]).reshape(8, 128).T
    bp[:, 168:176] = f32(inp["bs1"]).reshape(8, 128).T
    bp[0, 176] = float(np.asarray(inp["bs2"]).reshape(-1)[0])

    shared = {
        "biasp": bp,
        "wq": bf(inp["wq"]), "wk": bf(inp["wk"]), "wv": bf(inp["wv"]),
        "wo": bf(inp["wo"]), "w1": bf(inp["w1"]), "w2": bf(inp["w2"]),
        "ws0": bf(inp["ws0"]), "ws1": bf(inp["ws1"]), "ws2": bf(inp["ws2"]),
    }

    in_maps = []
    for c in range(NC_):
        sl = slice(c * SL, (c + 1) * SL)
        m = dict(shared)
        m["xT"] = np.ascontiguousarray(x0[sl].T).astype(np.float32)
        m["ropeT"] = np.ascontiguousarray(
            np.concatenate([cr[sl].T, sr[sl].T, cc[sl].T, sc[sl].T], axis=0)
        ).astype(np.float32)
        in_maps.append(m)
    return in_maps


def kernel(**inputs):
    nc = _get_built()
    in_maps = _host_prep(inputs)
    res = run_bass_kernel_spmd(nc, in_maps, core_ids=list(range(NC_)))
    logits = np.concatenate(
        [np.asarray(res.results[c]["logits"]).reshape(SL) for c in range(NC_)])
    return logits.astype(np.float32)


if __name__ == "__main__":
    data = np.load("/root/problem/ref_data.npz")
    expected = data["__expected"]
    inputs = {k: data[k] for k in data.files if k != "__expected"}
    got = kernel(**inputs)
    err = np.abs(got - expected)
    rel = np.linalg.norm(got - expected) / np.linalg.norm(expected)
    print("max abs err:", err.max(), "rel l2:", rel)
